# revision 55
# baseline (speedup 1.0000x reference)
"""JPEG layer (nn_JpegLayer) Trainium2 Bass kernel, 8-core data parallel.

Pipeline per image (per core: 4 images of [3,512,512]):
  P1: 3-accum f32r matmuls fold RGB->YCC color mix + H-DCT (+ vertical
      2x-pool for chroma) into [128,1024] 2-bank PSUM pairs. The Y drain
      (ACT Identity) subtracts sqrt(8)*L on h-freq DC rows = the -L level
      shift folded through the H-DCT.
  T1: PE transposes (f32r, identity rhs) -> [w, h-freq] pairs.
  P2: W-DCT (f32r). Chroma's 4 M=64 outputs pack into one [128,1024]
      pair via zero-padded [128,128] weight halves accumulated into the
      same region (the ISA rejects nonzero PSUM dst partition offsets).
  Q : all on DVE over [128,1024] pairs: e = d*(1/q) (TT, psum read,
      [128,512] table broadcast via stride-0 AP); r = (e + 1.5*2^23) -
      1.5*2^23 (dual-op tensor_scalar, bf16 out -- |r| < 256 so bf16 is
      exact); dec = r*q (bf16 TT).
  S3: fused W-IDCT + transpose as regular bf16 matmuls with dec chunks
      as the stationary operand (replaces P3 matmuls + T2 transposes).
      Chroma: 2x horizontal upsample folded into the streamed matrix,
      and both s-parity halves of a column range handled by one N=256
      matmul (they sit in different partition halves of the lhsT).
  t2y drain (ACT Identity) adds sqrt(8)*L on h-freq DC rows -> +L after
      the H-IDCT; t2c drain is a plain cast.
  P4: bf16 matmuls: H-IDCT + YCC->RGB fold (+ vertical upsample for
      chroma).
  out: clamp [0,1] via DVE dual-op tensor_scalar, bf16 store (halves
      output DMA traffic), host upcasts to f32.

Forward path (P1..Q input) stays f32r; only post-round data is bf16.
PSUM pooling: P1/T1/P2 rotate one 4-bank pool, S3/P4 the other, so
image i+1's P1 never waits on image i's late stages.
"""
import os
import sys
sys.path.insert(0, '/opt/trn_rl_repo')
import numpy as np
import ml_dtypes
import concourse.bacc as bacc
import concourse.bass as bass
import concourse.mybir as mybir
import concourse.tile as tile
from concourse import bass_utils

N_CORES = 8
IMG_PER_CORE = 4
H = W = 512
LEVEL = np.float32(128.0 / 255.0)
C_ROUND = 12582912.0   # 1.5*2^23: (x+C)-C == round-half-even(x)
F32 = mybir.dt.float32
F32R = mybir.dt.float32r
BF16 = mybir.dt.bfloat16

RGB2YCC = np.array([[0.299, 0.587, 0.114],
                    [-0.168735892, -0.331264108, 0.5],
                    [0.5, -0.418687589, -0.081312411]], dtype=np.float32)
CB_C = np.array([0.0, -0.344136286, 1.772], dtype=np.float32)
CR_C = np.array([1.402, -0.714136286, 0.0], dtype=np.float32)


def _dct8():
    i = np.arange(8)[:, None].astype(np.float64)
    j = np.arange(8)[None, :].astype(np.float64)
    m = np.sqrt(2.0 / 8) * np.cos(np.pi * (2 * j + 1) * i / 16.0)
    m[0, :] = 1.0 / np.sqrt(8.0)
    return m.astype(np.float32)


def _blockdiag(b, reps):
    r, c = b.shape
    out = np.zeros((r * reps, c * reps), dtype=np.float32)
    for k in range(reps):
        out[k * r:(k + 1) * r, k * c:(k + 1) * c] = b
    return out


def _build_consts(quantize):
    D = _dct8()
    BD_T = _blockdiag(D.T, 16)             # [128,128] fwd 1D-DCT as lhsT
    BD = _blockdiag(D, 16)                 # [128,128] inverse
    # pooled fwd: PF[2ii+dh, u] = D[u,ii]/2 per 16->8 block   [128, 64]
    pf8 = np.zeros((16, 8), dtype=np.float32)
    for ii in range(8):
        for dh in range(2):
            pf8[2 * ii + dh, :] = D[:, ii] * 0.5
    PF = _blockdiag(pf8, 8)                # [128, 64]
    # upsample inverse: pu8[v, 2jj+dw] = D[v,jj]   [64, 128]
    pu8 = np.zeros((8, 16), dtype=np.float32)
    for jj in range(8):
        for dw in range(2):
            pu8[:, 2 * jj + dw] = D[:, jj]
    PU = _blockdiag(pu8, 8)                # [64, 128]

    bf = ml_dtypes.bfloat16
    consts = {}
    for c in range(3):
        consts[f"w1y{c}"] = RGB2YCC[0, c] * BD_T
        consts[f"w1c{c}"] = np.concatenate(
            [RGB2YCC[1, c] * PF, RGB2YCC[2, c] * PF], axis=1)  # [128,128]
    consts["w2y"] = BD_T
    # chroma W-DCT halves: s-even fills psum partitions 0-63, s-odd 64-127,
    # via zero-padded weights accumulated into the same full-partition region
    # (the ISA rejects matmuls with a nonzero dst partition offset).
    w2c_lo = np.zeros((128, 128), dtype=np.float32)
    w2c_lo[:, 0:64] = PF
    w2c_hi = np.zeros((128, 128), dtype=np.float32)
    w2c_hi[:, 64:128] = PF
    consts["w2c_lo"] = w2c_lo
    consts["w2c_hi"] = w2c_hi
    consts["ident"] = np.eye(128, dtype=np.float32)
    # Y level shift as per-partition ACT biases on h-freq DC rows:
    # -sqrt(8)*L on the d1y drain (-L before the DCT pair), +sqrt(8)*L on
    # the t2y drain (+L after the IDCT pair).
    lneg = np.zeros((128, 1), dtype=np.float32)
    lneg[0::8, 0] = -np.float32(np.sqrt(8.0) * LEVEL)
    consts["lneg"] = lneg
    lpos = np.zeros((128, 1), dtype=np.float32)
    lpos[0::8, 0] = np.float32(np.sqrt(8.0) * LEVEL)
    consts["lpos"] = lpos

    # quant tables over [128,1024] pairs; q = round(quantize[0]*255)/255
    q = (np.round(quantize[0].astype(np.float32) * np.float32(255.0))
         / np.float32(255.0)).astype(np.float32)
    rq = (1.0 / q.astype(np.float64)).astype(np.float32)
    consts["rqt2"] = np.tile(rq.T, (16, 64)).astype(np.float32)   # [128,512]
    consts["qt2b"] = np.tile(q.T, (16, 64)).astype(bf)            # [128,512]

    consts["bdw_b"] = BD.astype(bf)        # S3-Y streamed matrix
    # S3-C streamed matrix: one N=256 matmul covers both s-chunks of a
    # column range (s-even w2-freqs sit in lhsT rows 0-63 -> w chunk left
    # half, s-odd in rows 64-127 -> right half)
    pud2 = np.zeros((128, 256), dtype=np.float32)
    pud2[0:64, 0:128] = PU
    pud2[64:128, 128:256] = PU
    consts["pud2"] = pud2.astype(bf)
    consts["w4y_b"] = BD.astype(bf)
    for name, cb, cr in (("R", CB_C[0], CR_C[0]), ("G", CB_C[1], CR_C[1]),
                         ("B", CB_C[2], CR_C[2])):
        m = np.zeros((128, 128), dtype=np.float32)
        m[0:64, :] = cb * PU
        m[64:128, :] = cr * PU
        consts[f"w4c{name}_b"] = m.astype(bf)
    return consts


_CONST_INFO = None


def _build_nc():
    nc = bacc.Bacc("TRN2", target_bir_lowering=False, debug=False,
                   enable_asserts=False, num_devices=N_CORES)
    x_d = nc.dram_tensor("x", [IMG_PER_CORE, 3, H, W], F32R,
                         kind="ExternalInput").ap()
    # bf16 output halves the store DMA traffic; host upcasts to f32
    out_d = nc.dram_tensor("out", [IMG_PER_CORE, 3, H, W], BF16,
                           kind="ExternalOutput").ap()
    cd = {}
    for name, (shape, dt) in _CONST_INFO.items():
        cd[name] = nc.dram_tensor(name, list(shape), dt,
                                  kind="ExternalInput").ap()

    ACT = mybir.ActivationFunctionType
    OP = mybir.AluOpType

    with tile.TileContext(nc) as tc:
        with tc.tile_pool(name="consts", bufs=1) as cp, \
             tc.tile_pool(name="xin", bufs=24) as xp, \
             tc.tile_pool(name="fwd", bufs=4) as fp, \
             tc.tile_pool(name="qnt", bufs=4) as qp, \
             tc.tile_pool(name="dcd", bufs=3) as dp, \
             tc.tile_pool(name="outp", bufs=8) as op_, \
             tc.tile_pool(name="psmm", bufs=2, space="PSUM") as pmm, \
             tc.tile_pool(name="pstp", bufs=2, space="PSUM") as ptp:

            # load only P1's weights before image 0; the rest of the consts
            # go out after image 0's input DMAs so the PE can start early
            # lneg/lpos (512B bias vectors) gate the very first P1 drains;
            # w2y gates P2 of image 0 — all must beat image 0's 3MB of
            # input. Alternate the small early consts across both HWDGE
            # rings so they don't serialize ahead of the input triggers.
            early = [n for n in _CONST_INFO
                     if n.startswith(("w1y", "w1c"))
                     or n in ("ident", "lneg", "lpos", "w2y")]
            cs = {}
            for i, name in enumerate(early):
                shape, dt = _CONST_INFO[name]
                cs[name] = cp.tile(list(shape), dt, tag=f"c_{name}",
                                   name=f"c_{name}")
                eng = nc.scalar if i % 2 else nc.sync
                eng.dma_start(cs[name][:], cd[name])

            for img in range(IMG_PER_CORE):
                # ---- load RGB planes (one DMA per channel half) ----
                X = {}
                for t in range(4):
                    for c in range(3):
                        xt = xp.tile([128, 512], F32R, tag="x",
                                     name=f"x_{img}_{c}_{t}")
                        # image 0's loads alternate across both HWDGE rings
                        # (sync + scalar) so streaming ramps up sooner; the
                        # scalar queue is idle until the first drains (~7us)
                        eng = nc.scalar if (img == 0 and (3 * t + c) % 2) \
                            else nc.sync
                        eng.dma_start(
                            xt[:], x_d[img, c, 128 * t:128 * (t + 1), :])
                        X[c, t] = xt[:]
                if img == 0:
                    for name, (shape, dt) in _CONST_INFO.items():
                        if name in early:
                            continue
                        cs[name] = cp.tile(list(shape), dt, tag=f"c_{name}",
                                           name=f"c_{name}")
                        nc.sync.dma_start(cs[name][:], cd[name])

                # ---- P1: color + H-DCT (+v-pool chroma), pairs over t ----
                # Y/C interleaved per j-half so the in-order PE queue never
                # parks ready chroma work behind Y-work waiting on the
                # second input half
                d1y, d1c = [], []
                for j in range(2):
                    psY = ptp.tile([128, 1024], F32, tag="tp", name="pstp")
                    for b in range(2):
                        t = 2 * j + b
                        for c in range(3):
                            nc.tensor.matmul(psY[:, 512 * b:512 * (b + 1)],
                                             cs[f"w1y{c}"][:], X[c, t],
                                             start=(c == 0), stop=(c == 2))
                    ty = fp.tile([128, 1024], F32R, tag="d1y",
                                 name=f"d1y_{img}_{j}")
                    nc.scalar.activation(ty[:], psY[:], ACT.Identity,
                                         bias=cs["lneg"][:])
                    d1y.append(ty)
                    psC = ptp.tile([128, 1024], F32, tag="tp", name="pstp")
                    for b in range(2):
                        t = 2 * j + b
                        for c in range(3):
                            nc.tensor.matmul(psC[:, 512 * b:512 * (b + 1)],
                                             cs[f"w1c{c}"][:], X[c, t],
                                             start=(c == 0), stop=(c == 2))
                    tcc = fp.tile([128, 1024], F32R, tag="d1c",
                                  name=f"d1c_{img}_{j}")
                    nc.scalar.activation(tcc[:], psC[:], ACT.Copy)
                    d1c.append(tcc)

                # ---- T1: PE transposes, pairs over s ----
                t1y, t1c = [], []
                for u in range(2):
                    pty = ptp.tile([128, 1024], F32R, tag="tp", name="pstp")
                    for b in range(2):
                        s = 2 * u + b
                        for t in range(4):
                            nc.tensor.transpose(
                                pty[:, 512 * b + 128 * t:512 * b + 128 * (t + 1)],
                                d1y[t // 2][:, 512 * (t % 2) + 128 * s:
                                            512 * (t % 2) + 128 * (s + 1)],
                                cs["ident"][:])
                    sy = fp.tile([128, 1024], F32R, tag="t1y",
                                 name=f"t1y_{img}_{u}")
                    nc.scalar.activation(sy[:], pty[:], ACT.Copy)
                    t1y.append(sy)
                for u in range(2):
                    ptc = ptp.tile([128, 1024], F32R, tag="tp", name="pstp")
                    for b in range(2):
                        s = 2 * u + b
                        for t in range(4):
                            nc.tensor.transpose(
                                ptc[:, 512 * b + 128 * t:512 * b + 128 * (t + 1)],
                                d1c[t // 2][:, 512 * (t % 2) + 128 * s:
                                            512 * (t % 2) + 128 * (s + 1)],
                                cs["ident"][:])
                    sc = fp.tile([128, 1024], F32R, tag="t1c",
                                 name=f"t1c_{img}_{u}")
                    nc.scalar.activation(sc[:], ptc[:], ACT.Copy)
                    t1c.append(sc)

                # ---- P2 + quantize (all DVE) ----
                decy = []
                for u in range(2):
                    ps = ptp.tile([128, 1024], F32, tag="tp", name="pstp")
                    for b in range(2):
                        nc.tensor.matmul(ps[:, 512 * b:512 * (b + 1)],
                                         cs["w2y"][:],
                                         t1y[u][:, 512 * b:512 * (b + 1)],
                                         start=True, stop=True)
                    ey = qp.tile([128, 1024], F32, tag="ey",
                                 name=f"ey_{img}_{u}")
                    ry = qp.tile([128, 1024], BF16, tag="ry",
                                 name=f"ry_{img}_{u}")
                    dy = dp.tile([128, 1024], BF16, tag="decy",
                                 name=f"decy_{img}_{u}")
                    if img == IMG_PER_CORE - 1:
                        # last image: half-granular quant so S3's per-half
                        # dec reads unblock earlier (shortens the tail)
                        for h in range(2):
                            sl = slice(512 * h, 512 * (h + 1))
                            nc.vector.tensor_tensor(ey[:, sl], ps[:, sl],
                                                    cs["rqt2"][:], OP.mult)
                            nc.vector.tensor_scalar(ry[:, sl], ey[:, sl],
                                                    C_ROUND, C_ROUND,
                                                    OP.add, OP.subtract)
                            nc.vector.tensor_tensor(dy[:, sl], ry[:, sl],
                                                    cs["qt2b"][:], OP.mult)
                    else:
                        nc.vector.tensor_tensor(
                            ey[:].rearrange("p (b w) -> p b w", b=2),
                            ps[:].rearrange("p (b w) -> p b w", b=2),
                            cs["rqt2"][:].unsqueeze(1)
                            .broadcast_to([128, 2, 512]), OP.mult)
                        nc.vector.tensor_scalar(ry[:], ey[:], C_ROUND,
                                                C_ROUND, OP.add, OP.subtract)
                        nc.vector.tensor_tensor(
                            dy[:].rearrange("p (b w) -> p b w", b=2),
                            ry[:].rearrange("p (b w) -> p b w", b=2),
                            cs["qt2b"][:].unsqueeze(1)
                            .broadcast_to([128, 2, 512]), OP.mult)
                    decy.append(dy)

                psc = ptp.tile([128, 1024], F32, tag="tp", name="pstp")
                for s in range(4):
                    nc.tensor.matmul(
                        psc[:, 512 * (s // 2):512 * (s // 2) + 512],
                        cs["w2c_hi" if s % 2 else "w2c_lo"][:],
                        t1c[s // 2][:, 512 * (s % 2):512 * (s % 2) + 512],
                        start=(s % 2 == 0), stop=(s % 2 == 1))
                ec = qp.tile([128, 1024], F32, tag="ey", name=f"ec_{img}")
                rc = qp.tile([128, 1024], BF16, tag="ry", name=f"rc_{img}")
                decc = dp.tile([128, 1024], BF16, tag="decc",
                               name=f"decc_{img}")
                if img == IMG_PER_CORE - 1:
                    for h in range(2):
                        sl = slice(512 * h, 512 * (h + 1))
                        nc.vector.tensor_tensor(ec[:, sl], psc[:, sl],
                                                cs["rqt2"][:], OP.mult)
                        nc.vector.tensor_scalar(rc[:, sl], ec[:, sl],
                                                C_ROUND, C_ROUND,
                                                OP.add, OP.subtract)
                        nc.vector.tensor_tensor(decc[:, sl], rc[:, sl],
                                                cs["qt2b"][:], OP.mult)
                else:
                    nc.vector.tensor_tensor(
                        ec[:].rearrange("p (b w) -> p b w", b=2),
                        psc[:].rearrange("p (b w) -> p b w", b=2),
                        cs["rqt2"][:].unsqueeze(1)
                        .broadcast_to([128, 2, 512]), OP.mult)
                    nc.vector.tensor_scalar(rc[:], ec[:], C_ROUND, C_ROUND,
                                            OP.add, OP.subtract)
                    nc.vector.tensor_tensor(
                        decc[:].rearrange("p (b w) -> p b w", b=2),
                        rc[:].rearrange("p (b w) -> p b w", b=2),
                        cs["qt2b"][:].unsqueeze(1)
                        .broadcast_to([128, 2, 512]), OP.mult)

                # ---- S3: fused W-IDCT + transpose (bf16 matmuls) ----
                t2y, t2c = [], []
                for v in range(2):
                    ps = pmm.tile([128, 1024], F32, tag="mm", name="psmm")
                    for b in range(2):
                        t = 2 * v + b
                        for s in range(4):
                            nc.tensor.matmul(
                                ps[:, 512 * b + 128 * s:512 * b + 128 * (s + 1)],
                                decy[s // 2][:, 512 * (s % 2) + 128 * t:
                                             512 * (s % 2) + 128 * (t + 1)],
                                cs["bdw_b"][:], start=True, stop=True)
                    sy = dp.tile([128, 1024], BF16, tag="t2y",
                                 name=f"t2y_{img}_{v}")
                    nc.scalar.activation(sy[:], ps[:], ACT.Identity,
                                         bias=cs["lpos"][:])
                    t2y.append(sy)
                for v in range(2):
                    ps = pmm.tile([128, 1024], F32, tag="mm", name="psmm")
                    for b in range(2):
                        t = 2 * v + b
                        for g in range(2):
                            nc.tensor.matmul(
                                ps[:, 512 * b + 256 * g:512 * b + 256 * (g + 1)],
                                decc[:, 512 * g + 128 * t:
                                     512 * g + 128 * (t + 1)],
                                cs["pud2"][:], start=True, stop=True)
                    sc = dp.tile([128, 1024], BF16, tag="t2c",
                                 name=f"t2c_{img}_{v}")
                    nc.scalar.activation(sc[:], ps[:], ACT.Copy)
                    t2c.append(sc)

                # ---- P4: H-IDCT + color + clamp + store ----
                for ci, cname in enumerate(("R", "G", "B")):
                    for v in range(2):
                        # last image: the fwd-path pool (tp) is idle by now,
                        # so alternate P4 pairs across both pools to double
                        # the rotation slots during the tail drain
                        if img == IMG_PER_CORE - 1 and (2 * ci + v) % 2:
                            ps = ptp.tile([128, 1024], F32, tag="tp",
                                          name="pstp")
                        else:
                            ps = pmm.tile([128, 1024], F32, tag="mm",
                                          name="psmm")
                        for b in range(2):
                            nc.tensor.matmul(
                                ps[:, 512 * b:512 * (b + 1)], cs["w4y_b"][:],
                                t2y[v][:, 512 * b:512 * (b + 1)],
                                start=True, stop=False)
                            nc.tensor.matmul(
                                ps[:, 512 * b:512 * (b + 1)],
                                cs[f"w4c{cname}_b"][:],
                                t2c[v][:, 512 * b:512 * (b + 1)],
                                start=False, stop=True)
                        og = op_.tile([128, 1024], BF16, tag="og",
                                      name=f"og_{img}_{ci}_{v}")
                        if img == IMG_PER_CORE - 1:
                            # last image: clamp+store per half so the first
                            # half streams out while the second clamps
                            for b in range(2):
                                sl = slice(512 * b, 512 * (b + 1))
                                nc.vector.tensor_scalar(og[:, sl], ps[:, sl],
                                                        0.0, 1.0,
                                                        OP.max, OP.min)
                                t = 2 * v + b
                                nc.sync.dma_start(
                                    out_d[img, ci, 128 * t:128 * (t + 1), :],
                                    og[:, sl])
                        else:
                            nc.vector.tensor_scalar(og[:], ps[:], 0.0, 1.0,
                                                    OP.max, OP.min)
                            nc.sync.dma_start(
                                out_d[img, ci, 256 * v:256 * (v + 1), :]
                                .rearrange("(b p) w -> p b w", b=2),
                                og[:].rearrange("p (b w) -> p b w", b=2))
    nc.compile()
    return nc


_NC_CACHE = None


def kernel(input, quantize):
    global _NC_CACHE, _CONST_INFO
    input = np.asarray(input, dtype=np.float32)
    quantize = np.asarray(quantize, dtype=np.float32)
    consts = _build_consts(quantize)
    if _CONST_INFO is None:
        _CONST_INFO = {}
        for k, v in consts.items():
            dt = BF16 if v.dtype == ml_dtypes.bfloat16 else (
                F32 if k in ("rqt2", "lneg", "lpos") else F32R)
            _CONST_INFO[k] = (v.shape, dt)
    if _NC_CACHE is None:
        _NC_CACHE = _build_nc()
    nc = _NC_CACHE

    in_maps = []
    for core in range(N_CORES):
        shard = np.ascontiguousarray(
            input[core * IMG_PER_CORE:(core + 1) * IMG_PER_CORE])
        m = {"x": shard}
        m.update(consts)
        in_maps.append(m)
    trace = bool(os.environ.get("JPEG_TRACE"))
    kw = {}
    if trace:
        kw["trace"] = True
        td = os.environ.get("JPEG_TRACE_DIR")
        if td:
            os.makedirs(td, exist_ok=True)
            kw["tmpdir"] = td
    res = bass_utils.run_bass_kernel_spmd(nc, in_maps,
                                          core_ids=list(range(N_CORES)), **kw)
    global LAST_RESULT
    LAST_RESULT = res
    out = np.concatenate(
        [np.asarray(res.results[i]["out"]) for i in range(N_CORES)], axis=0)
    return out.astype(np.float32)


LAST_RESULT = None


# revision 56
# speedup vs baseline: 1.0131x; 1.0131x over previous
"""JPEG layer (nn_JpegLayer) Trainium2 Bass kernel, 8-core data parallel.

Pipeline per image (per core: 4 images of [3,512,512]):
  P1: 3-accum f32r matmuls fold RGB->YCC color mix + H-DCT (+ vertical
      2x-pool for chroma) into [128,1024] 2-bank PSUM pairs. The Y drain
      (ACT Identity) subtracts sqrt(8)*L on h-freq DC rows = the -L level
      shift folded through the H-DCT.
  T1: PE transposes (f32r, identity rhs) -> [w, h-freq] pairs.
  P2: W-DCT (f32r). Chroma's 4 M=64 outputs pack into one [128,1024]
      pair via zero-padded [128,128] weight halves accumulated into the
      same region (the ISA rejects nonzero PSUM dst partition offsets).
  Q : all on DVE over [128,1024] pairs: e = d*(1/q) (TT, psum read,
      [128,512] table broadcast via stride-0 AP); r = (e + 1.5*2^23) -
      1.5*2^23 (dual-op tensor_scalar, bf16 out -- |r| < 256 so bf16 is
      exact); dec = r*q (bf16 TT).
  S3: fused W-IDCT + transpose as regular bf16 matmuls with dec chunks
      as the stationary operand (replaces P3 matmuls + T2 transposes).
      Chroma: 2x horizontal upsample folded into the streamed matrix,
      and both s-parity halves of a column range handled by one N=256
      matmul (they sit in different partition halves of the lhsT).
  t2y drain (ACT Identity) adds sqrt(8)*L on h-freq DC rows -> +L after
      the H-IDCT; t2c drain is a plain cast.
  P4: bf16 matmuls: H-IDCT + YCC->RGB fold (+ vertical upsample for
      chroma).
  out: clamp [0,1] via DVE dual-op tensor_scalar, bf16 store (halves
      output DMA traffic), host upcasts to f32.

Forward path (P1..Q input) stays f32r; only post-round data is bf16.
PSUM pooling: P1/T1/P2 rotate one 4-bank pool, S3/P4 the other, so
image i+1's P1 never waits on image i's late stages.
"""
import os
import sys
sys.path.insert(0, '/opt/trn_rl_repo')
import numpy as np
import ml_dtypes
import concourse.bacc as bacc
import concourse.bass as bass
import concourse.mybir as mybir
import concourse.tile as tile
from concourse import bass_utils

N_CORES = 8
IMG_PER_CORE = 4
H = W = 512
LEVEL = np.float32(128.0 / 255.0)
C_ROUND = 12582912.0   # 1.5*2^23: (x+C)-C == round-half-even(x)
F32 = mybir.dt.float32
F32R = mybir.dt.float32r
BF16 = mybir.dt.bfloat16

RGB2YCC = np.array([[0.299, 0.587, 0.114],
                    [-0.168735892, -0.331264108, 0.5],
                    [0.5, -0.418687589, -0.081312411]], dtype=np.float32)
CB_C = np.array([0.0, -0.344136286, 1.772], dtype=np.float32)
CR_C = np.array([1.402, -0.714136286, 0.0], dtype=np.float32)


def _dct8():
    i = np.arange(8)[:, None].astype(np.float64)
    j = np.arange(8)[None, :].astype(np.float64)
    m = np.sqrt(2.0 / 8) * np.cos(np.pi * (2 * j + 1) * i / 16.0)
    m[0, :] = 1.0 / np.sqrt(8.0)
    return m.astype(np.float32)


def _blockdiag(b, reps):
    r, c = b.shape
    out = np.zeros((r * reps, c * reps), dtype=np.float32)
    for k in range(reps):
        out[k * r:(k + 1) * r, k * c:(k + 1) * c] = b
    return out


def _build_consts(quantize):
    D = _dct8()
    BD_T = _blockdiag(D.T, 16)             # [128,128] fwd 1D-DCT as lhsT
    BD = _blockdiag(D, 16)                 # [128,128] inverse
    # pooled fwd: PF[2ii+dh, u] = D[u,ii]/2 per 16->8 block   [128, 64]
    pf8 = np.zeros((16, 8), dtype=np.float32)
    for ii in range(8):
        for dh in range(2):
            pf8[2 * ii + dh, :] = D[:, ii] * 0.5
    PF = _blockdiag(pf8, 8)                # [128, 64]
    # upsample inverse: pu8[v, 2jj+dw] = D[v,jj]   [64, 128]
    pu8 = np.zeros((8, 16), dtype=np.float32)
    for jj in range(8):
        for dw in range(2):
            pu8[:, 2 * jj + dw] = D[:, jj]
    PU = _blockdiag(pu8, 8)                # [64, 128]

    bf = ml_dtypes.bfloat16
    consts = {}
    for c in range(3):
        consts[f"w1y{c}"] = RGB2YCC[0, c] * BD_T
        consts[f"w1c{c}"] = np.concatenate(
            [RGB2YCC[1, c] * PF, RGB2YCC[2, c] * PF], axis=1)  # [128,128]
    consts["w2y"] = BD_T
    # chroma W-DCT halves: s-even fills psum partitions 0-63, s-odd 64-127,
    # via zero-padded weights accumulated into the same full-partition region
    # (the ISA rejects matmuls with a nonzero dst partition offset).
    w2c_lo = np.zeros((128, 128), dtype=np.float32)
    w2c_lo[:, 0:64] = PF
    w2c_hi = np.zeros((128, 128), dtype=np.float32)
    w2c_hi[:, 64:128] = PF
    consts["w2c_lo"] = w2c_lo
    consts["w2c_hi"] = w2c_hi
    consts["ident"] = np.eye(128, dtype=np.float32)
    # Y level shift as per-partition ACT biases on h-freq DC rows:
    # -sqrt(8)*L on the d1y drain (-L before the DCT pair), +sqrt(8)*L on
    # the t2y drain (+L after the IDCT pair).
    lneg = np.zeros((128, 1), dtype=np.float32)
    lneg[0::8, 0] = -np.float32(np.sqrt(8.0) * LEVEL)
    consts["lneg"] = lneg
    lpos = np.zeros((128, 1), dtype=np.float32)
    lpos[0::8, 0] = np.float32(np.sqrt(8.0) * LEVEL)
    consts["lpos"] = lpos

    # quant tables over [128,1024] pairs; q = round(quantize[0]*255)/255
    q = (np.round(quantize[0].astype(np.float32) * np.float32(255.0))
         / np.float32(255.0)).astype(np.float32)
    rq = (1.0 / q.astype(np.float64)).astype(np.float32)
    consts["rqt2"] = np.tile(rq.T, (16, 64)).astype(np.float32)   # [128,512]
    consts["qt2b"] = np.tile(q.T, (16, 64)).astype(bf)            # [128,512]

    consts["bdw_b"] = BD.astype(bf)        # S3-Y streamed matrix
    # S3-C streamed matrix: one N=256 matmul covers both s-chunks of a
    # column range (s-even w2-freqs sit in lhsT rows 0-63 -> w chunk left
    # half, s-odd in rows 64-127 -> right half)
    pud2 = np.zeros((128, 256), dtype=np.float32)
    pud2[0:64, 0:128] = PU
    pud2[64:128, 128:256] = PU
    consts["pud2"] = pud2.astype(bf)
    consts["w4y_b"] = BD.astype(bf)
    for name, cb, cr in (("R", CB_C[0], CR_C[0]), ("G", CB_C[1], CR_C[1]),
                         ("B", CB_C[2], CR_C[2])):
        m = np.zeros((128, 128), dtype=np.float32)
        m[0:64, :] = cb * PU
        m[64:128, :] = cr * PU
        consts[f"w4c{name}_b"] = m.astype(bf)
    return consts


_CONST_INFO = None


def _build_nc():
    nc = bacc.Bacc("TRN2", target_bir_lowering=False, debug=False,
                   enable_asserts=False, num_devices=N_CORES)
    x_d = nc.dram_tensor("x", [IMG_PER_CORE, 3, H, W], F32R,
                         kind="ExternalInput").ap()
    # bf16 output halves the store DMA traffic; host upcasts to f32
    out_d = nc.dram_tensor("out", [IMG_PER_CORE, 3, H, W], BF16,
                           kind="ExternalOutput").ap()
    cd = {}
    for name, (shape, dt) in _CONST_INFO.items():
        cd[name] = nc.dram_tensor(name, list(shape), dt,
                                  kind="ExternalInput").ap()

    ACT = mybir.ActivationFunctionType
    OP = mybir.AluOpType

    with tile.TileContext(nc) as tc:
        with tc.tile_pool(name="consts", bufs=1) as cp, \
             tc.tile_pool(name="xin", bufs=24) as xp, \
             tc.tile_pool(name="fwd", bufs=4) as fp, \
             tc.tile_pool(name="qnt", bufs=4) as qp, \
             tc.tile_pool(name="dcd", bufs=3) as dp, \
             tc.tile_pool(name="outp", bufs=8) as op_, \
             tc.tile_pool(name="psmm", bufs=2, space="PSUM") as pmm, \
             tc.tile_pool(name="pstp", bufs=2, space="PSUM") as ptp:

            # load only P1's weights before image 0; the rest of the consts
            # go out after image 0's input DMAs so the PE can start early
            early = [n for n in _CONST_INFO
                     if n.startswith(("w1y", "w1c")) or n == "ident"]
            cs = {}
            for name in early:
                shape, dt = _CONST_INFO[name]
                cs[name] = cp.tile(list(shape), dt, tag=f"c_{name}",
                                   name=f"c_{name}")
                nc.sync.dma_start(cs[name][:], cd[name])

            for img in range(IMG_PER_CORE):
                # ---- load RGB planes (one DMA per channel half) ----
                X = {}
                for t in range(4):
                    for c in range(3):
                        xt = xp.tile([128, 512], F32R, tag="x",
                                     name=f"x_{img}_{c}_{t}")
                        # image 0's loads alternate across both HWDGE rings
                        # (sync + scalar) so streaming ramps up sooner; the
                        # scalar queue is idle until the first drains (~7us)
                        eng = nc.scalar if (img == 0 and (3 * t + c) % 2) \
                            else nc.sync
                        eng.dma_start(
                            xt[:], x_d[img, c, 128 * t:128 * (t + 1), :])
                        X[c, t] = xt[:]
                if img == 0:
                    for name, (shape, dt) in _CONST_INFO.items():
                        if name in early:
                            continue
                        cs[name] = cp.tile(list(shape), dt, tag=f"c_{name}",
                                           name=f"c_{name}")
                        nc.sync.dma_start(cs[name][:], cd[name])

                # ---- P1: color + H-DCT (+v-pool chroma), pairs over t ----
                # Y/C interleaved per j-half so the in-order PE queue never
                # parks ready chroma work behind Y-work waiting on the
                # second input half
                d1y, d1c = [], []
                for j in range(2):
                    psY = ptp.tile([128, 1024], F32, tag="tp", name="pstp")
                    for b in range(2):
                        t = 2 * j + b
                        for c in range(3):
                            nc.tensor.matmul(psY[:, 512 * b:512 * (b + 1)],
                                             cs[f"w1y{c}"][:], X[c, t],
                                             start=(c == 0), stop=(c == 2))
                    ty = fp.tile([128, 1024], F32R, tag="d1y",
                                 name=f"d1y_{img}_{j}")
                    nc.scalar.activation(ty[:], psY[:], ACT.Identity,
                                         bias=cs["lneg"][:])
                    d1y.append(ty)
                    psC = ptp.tile([128, 1024], F32, tag="tp", name="pstp")
                    for b in range(2):
                        t = 2 * j + b
                        for c in range(3):
                            nc.tensor.matmul(psC[:, 512 * b:512 * (b + 1)],
                                             cs[f"w1c{c}"][:], X[c, t],
                                             start=(c == 0), stop=(c == 2))
                    tcc = fp.tile([128, 1024], F32R, tag="d1c",
                                  name=f"d1c_{img}_{j}")
                    nc.scalar.activation(tcc[:], psC[:], ACT.Copy)
                    d1c.append(tcc)

                # ---- T1: PE transposes, pairs over s ----
                t1y, t1c = [], []
                for u in range(2):
                    pty = ptp.tile([128, 1024], F32R, tag="tp", name="pstp")
                    for b in range(2):
                        s = 2 * u + b
                        for t in range(4):
                            nc.tensor.transpose(
                                pty[:, 512 * b + 128 * t:512 * b + 128 * (t + 1)],
                                d1y[t // 2][:, 512 * (t % 2) + 128 * s:
                                            512 * (t % 2) + 128 * (s + 1)],
                                cs["ident"][:])
                    sy = fp.tile([128, 1024], F32R, tag="t1y",
                                 name=f"t1y_{img}_{u}")
                    nc.scalar.activation(sy[:], pty[:], ACT.Copy)
                    t1y.append(sy)
                for u in range(2):
                    ptc = ptp.tile([128, 1024], F32R, tag="tp", name="pstp")
                    for b in range(2):
                        s = 2 * u + b
                        for t in range(4):
                            nc.tensor.transpose(
                                ptc[:, 512 * b + 128 * t:512 * b + 128 * (t + 1)],
                                d1c[t // 2][:, 512 * (t % 2) + 128 * s:
                                            512 * (t % 2) + 128 * (s + 1)],
                                cs["ident"][:])
                    sc = fp.tile([128, 1024], F32R, tag="t1c",
                                 name=f"t1c_{img}_{u}")
                    nc.scalar.activation(sc[:], ptc[:], ACT.Copy)
                    t1c.append(sc)

                # ---- P2 + quantize (all DVE) ----
                decy = []
                for u in range(2):
                    ps = ptp.tile([128, 1024], F32, tag="tp", name="pstp")
                    for b in range(2):
                        nc.tensor.matmul(ps[:, 512 * b:512 * (b + 1)],
                                         cs["w2y"][:],
                                         t1y[u][:, 512 * b:512 * (b + 1)],
                                         start=True, stop=True)
                    ey = qp.tile([128, 1024], F32, tag="ey",
                                 name=f"ey_{img}_{u}")
                    ry = qp.tile([128, 1024], BF16, tag="ry",
                                 name=f"ry_{img}_{u}")
                    dy = dp.tile([128, 1024], BF16, tag="decy",
                                 name=f"decy_{img}_{u}")
                    if img == IMG_PER_CORE - 1:
                        # last image: half-granular quant so S3's per-half
                        # dec reads unblock earlier (shortens the tail)
                        for h in range(2):
                            sl = slice(512 * h, 512 * (h + 1))
                            nc.vector.tensor_tensor(ey[:, sl], ps[:, sl],
                                                    cs["rqt2"][:], OP.mult)
                            nc.vector.tensor_scalar(ry[:, sl], ey[:, sl],
                                                    C_ROUND, C_ROUND,
                                                    OP.add, OP.subtract)
                            nc.vector.tensor_tensor(dy[:, sl], ry[:, sl],
                                                    cs["qt2b"][:], OP.mult)
                    else:
                        nc.vector.tensor_tensor(
                            ey[:].rearrange("p (b w) -> p b w", b=2),
                            ps[:].rearrange("p (b w) -> p b w", b=2),
                            cs["rqt2"][:].unsqueeze(1)
                            .broadcast_to([128, 2, 512]), OP.mult)
                        nc.vector.tensor_scalar(ry[:], ey[:], C_ROUND,
                                                C_ROUND, OP.add, OP.subtract)
                        nc.vector.tensor_tensor(
                            dy[:].rearrange("p (b w) -> p b w", b=2),
                            ry[:].rearrange("p (b w) -> p b w", b=2),
                            cs["qt2b"][:].unsqueeze(1)
                            .broadcast_to([128, 2, 512]), OP.mult)
                    decy.append(dy)

                psc = ptp.tile([128, 1024], F32, tag="tp", name="pstp")
                for s in range(4):
                    nc.tensor.matmul(
                        psc[:, 512 * (s // 2):512 * (s // 2) + 512],
                        cs["w2c_hi" if s % 2 else "w2c_lo"][:],
                        t1c[s // 2][:, 512 * (s % 2):512 * (s % 2) + 512],
                        start=(s % 2 == 0), stop=(s % 2 == 1))
                ec = qp.tile([128, 1024], F32, tag="ey", name=f"ec_{img}")
                rc = qp.tile([128, 1024], BF16, tag="ry", name=f"rc_{img}")
                decc = dp.tile([128, 1024], BF16, tag="decc",
                               name=f"decc_{img}")
                if img == IMG_PER_CORE - 1:
                    for h in range(2):
                        sl = slice(512 * h, 512 * (h + 1))
                        nc.vector.tensor_tensor(ec[:, sl], psc[:, sl],
                                                cs["rqt2"][:], OP.mult)
                        nc.vector.tensor_scalar(rc[:, sl], ec[:, sl],
                                                C_ROUND, C_ROUND,
                                                OP.add, OP.subtract)
                        nc.vector.tensor_tensor(decc[:, sl], rc[:, sl],
                                                cs["qt2b"][:], OP.mult)
                else:
                    nc.vector.tensor_tensor(
                        ec[:].rearrange("p (b w) -> p b w", b=2),
                        psc[:].rearrange("p (b w) -> p b w", b=2),
                        cs["rqt2"][:].unsqueeze(1)
                        .broadcast_to([128, 2, 512]), OP.mult)
                    nc.vector.tensor_scalar(rc[:], ec[:], C_ROUND, C_ROUND,
                                            OP.add, OP.subtract)
                    nc.vector.tensor_tensor(
                        decc[:].rearrange("p (b w) -> p b w", b=2),
                        rc[:].rearrange("p (b w) -> p b w", b=2),
                        cs["qt2b"][:].unsqueeze(1)
                        .broadcast_to([128, 2, 512]), OP.mult)

                # ---- S3: fused W-IDCT + transpose (bf16 matmuls) ----
                t2y, t2c = [], []
                for v in range(2):
                    ps = pmm.tile([128, 1024], F32, tag="mm", name="psmm")
                    for b in range(2):
                        t = 2 * v + b
                        for s in range(4):
                            nc.tensor.matmul(
                                ps[:, 512 * b + 128 * s:512 * b + 128 * (s + 1)],
                                decy[s // 2][:, 512 * (s % 2) + 128 * t:
                                             512 * (s % 2) + 128 * (t + 1)],
                                cs["bdw_b"][:], start=True, stop=True)
                    sy = dp.tile([128, 1024], BF16, tag="t2y",
                                 name=f"t2y_{img}_{v}")
                    nc.scalar.activation(sy[:], ps[:], ACT.Identity,
                                         bias=cs["lpos"][:])
                    t2y.append(sy)
                for v in range(2):
                    ps = pmm.tile([128, 1024], F32, tag="mm", name="psmm")
                    for b in range(2):
                        t = 2 * v + b
                        for g in range(2):
                            nc.tensor.matmul(
                                ps[:, 512 * b + 256 * g:512 * b + 256 * (g + 1)],
                                decc[:, 512 * g + 128 * t:
                                     512 * g + 128 * (t + 1)],
                                cs["pud2"][:], start=True, stop=True)
                    sc = dp.tile([128, 1024], BF16, tag="t2c",
                                 name=f"t2c_{img}_{v}")
                    nc.scalar.activation(sc[:], ps[:], ACT.Copy)
                    t2c.append(sc)

                # ---- P4: H-IDCT + color + clamp + store ----
                for ci, cname in enumerate(("R", "G", "B")):
                    for v in range(2):
                        # last image: the fwd-path pool (tp) is idle by now,
                        # so alternate P4 pairs across both pools to double
                        # the rotation slots during the tail drain
                        if img == IMG_PER_CORE - 1 and (2 * ci + v) % 2:
                            ps = ptp.tile([128, 1024], F32, tag="tp",
                                          name="pstp")
                        else:
                            ps = pmm.tile([128, 1024], F32, tag="mm",
                                          name="psmm")
                        for b in range(2):
                            nc.tensor.matmul(
                                ps[:, 512 * b:512 * (b + 1)], cs["w4y_b"][:],
                                t2y[v][:, 512 * b:512 * (b + 1)],
                                start=True, stop=False)
                            nc.tensor.matmul(
                                ps[:, 512 * b:512 * (b + 1)],
                                cs[f"w4c{cname}_b"][:],
                                t2c[v][:, 512 * b:512 * (b + 1)],
                                start=False, stop=True)
                        og = op_.tile([128, 1024], BF16, tag="og",
                                      name=f"og_{img}_{ci}_{v}")
                        if img == IMG_PER_CORE - 1:
                            # last image: clamp+store per half so the first
                            # half streams out while the second clamps
                            for b in range(2):
                                sl = slice(512 * b, 512 * (b + 1))
                                nc.vector.tensor_scalar(og[:, sl], ps[:, sl],
                                                        0.0, 1.0,
                                                        OP.max, OP.min)
                                t = 2 * v + b
                                nc.sync.dma_start(
                                    out_d[img, ci, 128 * t:128 * (t + 1), :],
                                    og[:, sl])
                        else:
                            nc.vector.tensor_scalar(og[:], ps[:], 0.0, 1.0,
                                                    OP.max, OP.min)
                            nc.sync.dma_start(
                                out_d[img, ci, 256 * v:256 * (v + 1), :]
                                .rearrange("(b p) w -> p b w", b=2),
                                og[:].rearrange("p (b w) -> p b w", b=2))
    nc.compile()
    return nc


_NC_CACHE = None


def kernel(input, quantize):
    global _NC_CACHE, _CONST_INFO
    input = np.asarray(input, dtype=np.float32)
    quantize = np.asarray(quantize, dtype=np.float32)
    consts = _build_consts(quantize)
    if _CONST_INFO is None:
        _CONST_INFO = {}
        for k, v in consts.items():
            dt = BF16 if v.dtype == ml_dtypes.bfloat16 else (
                F32 if k in ("rqt2", "lneg", "lpos") else F32R)
            _CONST_INFO[k] = (v.shape, dt)
    if _NC_CACHE is None:
        _NC_CACHE = _build_nc()
    nc = _NC_CACHE

    in_maps = []
    for core in range(N_CORES):
        shard = np.ascontiguousarray(
            input[core * IMG_PER_CORE:(core + 1) * IMG_PER_CORE])
        m = {"x": shard}
        m.update(consts)
        in_maps.append(m)
    trace = bool(os.environ.get("JPEG_TRACE"))
    kw = {}
    if trace:
        kw["trace"] = True
        td = os.environ.get("JPEG_TRACE_DIR")
        if td:
            os.makedirs(td, exist_ok=True)
            kw["tmpdir"] = td
    res = bass_utils.run_bass_kernel_spmd(nc, in_maps,
                                          core_ids=list(range(N_CORES)), **kw)
    global LAST_RESULT
    LAST_RESULT = res
    out = np.concatenate(
        [np.asarray(res.results[i]["out"]) for i in range(N_CORES)], axis=0)
    return out.astype(np.float32)


LAST_RESULT = None
